# revision 47
# baseline (speedup 1.0000x reference)
"""DKVMN forward kernel for 8 Trainium2 NeuronCores (v2).

Data-parallel over batch: B=128 -> 16 per core, split into 2 groups of
8 rows. Per-group state v [d=128 partitions, (m,b)=50*8=400 free] bf16.
(m,b) column order (col = m*8 + b) makes the e/a gate broadcasts have
stride-1 last dims so DVE tensor_tensor runs in 2x 16-bit mode.

Per group-step (slot s = 2t+g), engine assignment:
  w_ps  = sel_t.T @ attnb_g      PE (bf16) -> PSUM f32   (prefetch 2 slots)
  w_sb  = bf16(w_ps)             ACT copy
  z     = v * w_sb               DVE 2x
  zf    = z[:,:200]+z[:,200:]    GPSIMD (fold m 50->25)
  read  = reduce_m(zf) bf16      DVE (strided last dim, 1x)
  hps   = w1r@read + w1q@qe      PE
  th    = tanh(hps+b1) bf16      ACT
  e     = sigmoid(w2er@th+eb)    PE+ACT, bf16
  a     = tanh(w2ad@th+ab)       PE+ACT, bf16
  t1    = z * bc(e)              DVE 2x
  u     = (t1 * -1) + v          GPSIMD scalar_tensor_tensor
  t2    = w_sb * bc(a)           DVE 2x
  v'    = u + t2                 DVE 2x

Slot schedule "U(s-2); R(s); G(s)" keeps ~4 independent DVE ops queued
between a group's reduce and its gate-dependent t1, hiding the ACT/PE
gate latency.
"""

import os
import numpy as np
import ml_dtypes
from contextlib import ExitStack

import concourse.bass as bass
import concourse.bacc as bacc
import concourse.mybir as mybir
import concourse.tile as tile
import concourse.bass_utils as bass_utils
from concourse.masks import make_identity

B, S, M, D, NQ = 128, 100, 50, 128, 10000
NCORES = 8
BC = B // NCORES          # 16 batch rows per core
GB = 8                    # rows per group
GW = M * GB               # 400 group state width
HW = GW // 2              # 200 fold width
NQTILES = (S * BC + 127) // 128   # 13 gather tiles
QCOLS = NQTILES * 128     # 1664

F32 = mybir.dt.float32
BF16 = mybir.dt.bfloat16
I32 = mybir.dt.int32
AF = mybir.ActivationFunctionType
OP = mybir.AluOpType
AX = mybir.AxisListType

_CACHE = {}


def _build_program():
    if "nc" in _CACHE:
        return _CACHE["nc"]

    nc = bacc.Bacc("TRN2", target_bir_lowering=False, debug=False,
                   enable_asserts=False, num_devices=NCORES)

    dram_in = {}
    for name, shape, dt in [
        ("qtb", [D, QCOLS], BF16),
        ("qlast", [D, BC], F32),
        ("kTb", [D, M], BF16),
        ("w1r", [D, D], BF16), ("w1q", [D, D], BF16),
        ("w2er", [D, D], BF16), ("w2ad", [D, D], BF16),
        ("b1", [D, 1], F32), ("eb", [D, 1], F32), ("ab", [D, 1], F32),
        ("ow1r", [D, D], F32), ("ow1q", [D, D], F32),
        ("ob1", [D, 1], F32), ("ow2", [D, 1], F32), ("ob2", [1, 1], F32),
    ]:
        dram_in[name] = nc.dram_tensor(name, shape, dt, kind="ExternalInput").ap()
    pred_out = nc.dram_tensor("pred", [1, BC], F32, kind="ExternalOutput").ap()

    with tile.TileContext(nc) as tc, ExitStack() as ctx:
        persist = ctx.enter_context(tc.tile_pool(name="persist", bufs=1))

        # ---- persistent SBUF tiles ----
        kTb = persist.tile([D, M], BF16, tag="kTb")
        w1r = persist.tile([D, D], BF16, tag="w1r")
        w1q = persist.tile([D, D], BF16, tag="w1q")
        w2er = persist.tile([D, D], BF16, tag="w2er")
        w2ad = persist.tile([D, D], BF16, tag="w2ad")
        b1 = persist.tile([D, 1], F32, tag="b1")
        eb = persist.tile([D, 1], F32, tag="eb")
        ab = persist.tile([D, 1], F32, tag="ab")
        ow1r = persist.tile([D, D], F32, tag="ow1r")
        ow1q = persist.tile([D, D], F32, tag="ow1q")
        ob1 = persist.tile([D, 1], F32, tag="ob1")
        ow2 = persist.tile([D, 1], F32, tag="ow2")
        ob2 = persist.tile([1, 1], F32, tag="ob2")
        ident = persist.tile([128, 128], F32, tag="ident")
        identb = persist.tile([128, 128], BF16, tag="identb")
        qlast = persist.tile([D, BC], F32, tag="qlast")
        qTb = persist.tile([D, QCOLS], BF16, tag="qTb")
        attn = persist.tile([S, 2 * GW], F32, tag="attn")
        attnb = persist.tile([S, 2 * GW], BF16, tag="attnb")
        vpp = [[persist.tile([D, GW], BF16, name=f"v{g}p{p}", tag=f"v{g}p{p}")
                for p in (0, 1)] for g in (0, 1)]

        # qtb (the big one) split across three DGE queues; spread the
        # small weight loads across the other engines' DGE queues.
        TH = QCOLS // 4
        nc.sync.dma_start(qTb[:, 0:2 * TH], dram_in["qtb"][:, 0:2 * TH])
        nc.scalar.dma_start(qTb[:, 2 * TH:3 * TH],
                            dram_in["qtb"][:, 2 * TH:3 * TH])
        nc.gpsimd.dma_start(qTb[:, 3 * TH:QCOLS],
                            dram_in["qtb"][:, 3 * TH:QCOLS])
        for i, (nm, t) in enumerate([
                ("kTb", kTb), ("w1r", w1r), ("w1q", w1q),
                ("w2er", w2er), ("w2ad", w2ad), ("b1", b1),
                ("eb", eb), ("ab", ab), ("ow1r", ow1r),
                ("ow1q", ow1q), ("ob1", ob1), ("ow2", ow2),
                ("ob2", ob2), ("qlast", qlast)]):
            eng = (nc.scalar, nc.gpsimd)[i % 2]
            eng.dma_start(t[:], dram_in[nm][:])
        make_identity(nc, ident[:])
        nc.vector.tensor_copy(identb[:], ident[:])
        nc.vector.memset(vpp[0][0][:], 0.0)
        nc.vector.memset(vpp[1][0][:], 0.0)

        # ---- phase 2: scores + softmax -> attn[s, (b,m)] f32 ----
        with tc.tile_pool(name="spsum", bufs=4, space="PSUM") as spsum:
            for b in range(BC):
                sc = spsum.tile([S, M], F32, tag="sc")
                qTsl = qTb[:, b:S * BC:BC]        # [128, 100] strided (s,b)
                nc.tensor.matmul(sc[:], qTsl, kTb[:], start=True, stop=True)
                if b % 2 == 0:
                    nc.vector.tensor_copy(attn[:, b * M:(b + 1) * M], sc[:])
                else:
                    nc.scalar.copy(attn[:, b * M:(b + 1) * M], sc[:])

        with tc.tile_pool(name="smx", bufs=1) as smx:
            a3 = attn[:].rearrange("p (b m) -> p b m", b=BC)
            mx = smx.tile([S, BC], F32, tag="mx")
            nc.vector.tensor_reduce(mx[:], a3, axis=AX.X, op=OP.max)
            mxb = mx[:, :, None].broadcast_to([S, BC, M])
            nc.vector.tensor_tensor(a3, a3, mxb, op=OP.subtract)
            nc.scalar.activation(attn[:], attn[:], AF.Exp)
            sm = smx.tile([S, BC], F32, tag="sm")
            nc.vector.tensor_reduce(sm[:], a3, axis=AX.X, op=OP.add)
            rec = smx.tile([S, BC], F32, tag="rec")
            nc.vector.reciprocal(rec[:], sm[:])
            recb = rec[:, :, None].broadcast_to([S, BC, M])
            nc.vector.tensor_tensor(a3, a3, recb, op=OP.mult)
            # reshuffle (b,m) f32 -> per-group (m,b) bf16
            src = attn[:].rearrange("p (g b m) -> p g b m", g=2, b=GB)
            dst = attnb[:].rearrange("p (g m b) -> p g b m", g=2, m=M)
            nc.vector.tensor_copy(dst, src)

        # ---- phase 3: the scan ----
        with tc.tile_pool(name="wps", bufs=3, space="PSUM") as wps, \
             tc.tile_pool(name="wsb", bufs=6) as wsbp, \
             tc.tile_pool(name="zp", bufs=3) as zp, \
             tc.tile_pool(name="zfp", bufs=3) as zfp, \
             tc.tile_pool(name="wide", bufs=8) as wide, \
             tc.tile_pool(name="small", bufs=16) as small, \
             tc.tile_pool(name="mlp", bufs=2, space="PSUM") as mlpp, \
             tc.tile_pool(name="fin", bufs=1, space="PSUM") as finp:

            NS = 2 * S  # slots
            FOLD_DVE = bool(int(os.environ.get("DKVMN_FOLD_DVE", "1")))
            STAGGER = bool(int(os.environ.get("DKVMN_STAGGER", "0")))
            wtile = [None] * NS     # w_sb bf16 tiles
            hqe = [None] * NS       # prefetched W1q@qe PSUM tiles
            state = [None] * NS     # (z, w_sb, e, a) per slot
            estore = [None] * NS    # e tiles kept for stagger dummies

            def emit_w(s):
                t, g = s // 2, s % 2
                sel = identb[0:S, t:t + 1].broadcast_to([S, D])
                wp = wps.tile([D, GW], F32, tag="wp")
                nc.tensor.matmul(wp[:], sel, attnb[:, g * GW:(g + 1) * GW],
                                 start=True, stop=True)
                w = wsbp.tile([D, GW], BF16, tag="w")
                nc.scalar.copy(w[:], wp[:])
                wtile[s] = w

            def emit_qe(s):
                t, g = s // 2, s % 2
                qeT = qTb[:, t * BC + g * GB: t * BC + (g + 1) * GB]
                gps = mlpp.tile([D, 3 * GB], F32, tag="hps")
                nc.tensor.matmul(gps[:, 0:GB], w1q[:], qeT, start=True,
                                 stop=False)
                hqe[s] = gps

            def emit_read_gates(s):
                t, g = s // 2, s % 2
                w = wtile[s]
                vcur = vpp[g][t % 2]
                if STAGGER and g == 1 and s + 1 < NS and \
                        estore[s + 1] is not None:
                    # Tiny op that reads group 0's NEXT-step e tile and
                    # rewrites one element of group 1's v unchanged
                    # (Identity(e*0 + v)): forces group 1's z to wait for
                    # group 0's step-(t+1) gates, staggering the two
                    # pipelines into anti-phase.
                    nc.scalar.activation(
                        vcur[0:1, 0:1], estore[s + 1][0:1, 0:1], AF.Identity,
                        bias=vcur[0:1, 0:1], scale=0.0)
                # z and the fused read+gate matmul in pipelined halves:
                # W1r @ read = sum_m (W1r @ z[:,m]) via stride-0 PSUM output
                # APs (start=False writes accumulate by address), with the
                # second z half computed while PE streams the first.
                z = zp.tile([D, GW], BF16, tag="z")
                gps = hqe[s]
                hqe[s] = None
                hbc = gps[:, None, 0:GB].broadcast_to([D, M // 2, GB])
                nc.vector.tensor_tensor(z[:, 0:HW], vcur[:, 0:HW],
                                        w[:, 0:HW], op=OP.mult)
                nc.tensor.matmul(
                    hbc, w1r[:],
                    z[:, 0:HW].rearrange("p (m b) -> p m b", m=M // 2),
                    start=False, stop=False)
                nc.vector.tensor_tensor(z[:, HW:GW], vcur[:, HW:GW],
                                        w[:, HW:GW], op=OP.mult)
                nc.tensor.matmul(
                    hbc, w1r[:],
                    z[:, HW:GW].rearrange("p (m b) -> p m b", m=M // 2),
                    start=False, stop=True)
                th = small.tile([D, GB], BF16, tag="th")
                nc.scalar.activation(th[:], gps[:, 0:GB], AF.Tanh, bias=b1[:])
                nc.tensor.matmul(gps[:, 2 * GB:3 * GB], w2ad[:], th[:],
                                 start=True, stop=True)
                a = small.tile([D, GB], BF16, tag="a")
                nc.scalar.activation(a[:], gps[:, 2 * GB:3 * GB], AF.Tanh,
                                     bias=ab[:])
                nc.tensor.matmul(gps[:, GB:2 * GB], w2er[:], th[:], start=True,
                                 stop=True)
                e = small.tile([D, GB], BF16, tag="e")
                nc.scalar.activation(e[:], gps[:, GB:2 * GB], AF.Sigmoid,
                                     bias=eb[:])
                state[s] = (z, w, e, a)
                estore[s] = e

            def emit_update(s):
                t, g = s // 2, s % 2
                z, w, e, a = state[s]
                vcur, vnext = vpp[g][t % 2], vpp[g][(t + 1) % 2]
                abc = a[:, None, :].broadcast_to([D, M // 2, GB])
                t2 = wide.tile([D, GW], BF16, tag="t2")
                q = wide.tile([D, GW], BF16, tag="q")
                for lo, hi in ((0, HW), (HW, GW)):
                    nc.vector.tensor_tensor(
                        t2[:, lo:hi].rearrange("p (m b) -> p m b", m=M // 2),
                        w[:, lo:hi].rearrange("p (m b) -> p m b", m=M // 2),
                        abc, op=OP.mult)
                    nc.vector.tensor_tensor(q[:, lo:hi], vcur[:, lo:hi],
                                            t2[:, lo:hi], op=OP.add)
                # t1 and v' in pipelined halves: v' of each half starts as
                # soon as its t1 half is done.
                ebc = e[:, None, :].broadcast_to([D, M // 2, GB])
                t1 = wide.tile([D, GW], BF16, tag="t1")
                for lo, hi in ((0, HW), (HW, GW)):
                    nc.vector.tensor_tensor(
                        t1[:, lo:hi].rearrange("p (m b) -> p m b", m=M // 2),
                        z[:, lo:hi].rearrange("p (m b) -> p m b", m=M // 2),
                        ebc, op=OP.mult)
                    nc.vector.tensor_tensor(vnext[:, lo:hi], q[:, lo:hi],
                                            t1[:, lo:hi], op=OP.subtract)
                state[s] = None
                if s < NS - 2:
                    wtile[s] = None    # keep the last step's w for the predict

            # Anti-phase slot order (0, 2, 1, 4, 3, ...): group 0 runs one
            # step ahead of group 1 so each group's gate latency is hidden
            # under the other group's DVE stretch instead of bunching.
            order = [0] + [x for k in range(1, S) for x in (2 * k, 2 * k - 1)] \
                + [NS - 1]
            emit_w(order[0]); emit_w(order[1])
            emit_qe(order[0])
            for i, s in enumerate(order):
                if s >= 2:
                    emit_update(s - 2)
                emit_read_gates(s)
                if i + 2 < NS:
                    emit_w(order[i + 2])
                if i + 1 < NS:
                    emit_qe(order[i + 1])
            emit_update(NS - 2)
            emit_update(NS - 1)

            # ---- final prediction (uses w from t=S-1, v after last update) ----
            readF = small.tile([D, BC], F32, tag="readF")
            for g in (0, 1):
                wf = wtile[2 * (S - 1) + g]
                zfin = zp.tile([D, GW], BF16, tag="z")
                nc.vector.tensor_tensor(zfin[:], vpp[g][S % 2][:], wf[:],
                                        op=OP.mult)
                nc.vector.tensor_reduce(
                    readF[:, g * GB:(g + 1) * GB],
                    zfin[:].rearrange("p (m b) -> p b m", b=GB),
                    axis=AX.X, op=OP.add)
            h2ps = finp.tile([D, BC], F32, tag="fin")
            nc.tensor.matmul(h2ps[:], ow1r[:], readF[:], start=True, stop=False)
            nc.tensor.matmul(h2ps[:], ow1q[:], qlast[:], start=False, stop=True)
            h2 = small.tile([D, BC], F32, tag="h2")
            nc.scalar.activation(h2[:], h2ps[:], AF.Relu, bias=ob1[:])
            pps = mlpp.tile([D, 3 * GB], F32, tag="hps")
            nc.tensor.matmul(pps[0:1, 0:BC], ow2[:], h2[:], start=True,
                             stop=True)
            ps = small.tile([1, BC], F32, tag="pred")
            nc.scalar.activation(ps[:], pps[0:1, 0:BC], AF.Sigmoid, bias=ob2[:])
            nc.sync.dma_start(pred_out[:], ps[:])

    nc.compile()
    _CACHE["nc"] = nc
    return nc


def _host_inputs(inputs):
    """Per-core input maps from the full problem inputs."""
    q = np.asarray(inputs["question_seq"]).astype(np.int64)
    emb = np.ascontiguousarray(np.asarray(inputs["emb"], dtype=np.float32))
    key_matrix = np.asarray(inputs["key_matrix"], dtype=np.float32)
    vu_w1 = np.asarray(inputs["vu_w1"], dtype=np.float32)
    vu_b1 = np.asarray(inputs["vu_b1"], dtype=np.float32)
    vu_w2 = np.asarray(inputs["vu_w2"], dtype=np.float32)
    vu_b2 = np.asarray(inputs["vu_b2"], dtype=np.float32)
    er_w = np.asarray(inputs["er_w"], dtype=np.float32)
    er_b = np.asarray(inputs["er_b"], dtype=np.float32)
    ad_w = np.asarray(inputs["ad_w"], dtype=np.float32)
    ad_b = np.asarray(inputs["ad_b"], dtype=np.float32)
    out_w1 = np.asarray(inputs["out_w1"], dtype=np.float32)
    out_b1 = np.asarray(inputs["out_b1"], dtype=np.float32)
    out_w2 = np.asarray(inputs["out_w2"], dtype=np.float32)
    out_b2 = np.asarray(inputs["out_b2"], dtype=np.float32)

    w2er = (vu_w2.astype(np.float64) @ er_w.astype(np.float64)).astype(np.float32)
    w2ad = (vu_w2.astype(np.float64) @ ad_w.astype(np.float64)).astype(np.float32)
    ebf = (vu_b2.astype(np.float64) @ er_w.astype(np.float64) + er_b).astype(np.float32)
    abf = (vu_b2.astype(np.float64) @ ad_w.astype(np.float64) + ad_b).astype(np.float32)

    bf = ml_dtypes.bfloat16
    shared = {
        "kTb": np.ascontiguousarray(key_matrix.T).astype(bf),
        "w1r": np.ascontiguousarray(vu_w1[:D]).astype(bf),
        "w1q": np.ascontiguousarray(vu_w1[D:]).astype(bf),
        "w2er": w2er.astype(bf), "w2ad": w2ad.astype(bf),
        "b1": vu_b1.reshape(D, 1), "eb": ebf.reshape(D, 1), "ab": abf.reshape(D, 1),
        "ow1r": np.ascontiguousarray(out_w1[:D]),
        "ow1q": np.ascontiguousarray(out_w1[D:]),
        "ob1": out_b1.reshape(D, 1),
        "ow2": np.ascontiguousarray(out_w2.reshape(D, 1)),
        "ob2": out_b2.reshape(1, 1),
    }
    in_maps = []
    for c in range(NCORES):
        qc = q[c * BC:(c + 1) * BC, :]          # [BC, S]
        idxs = qc.T.reshape(-1)                  # n = s*BC + b order
        qg = emb[idxs]                           # [S*BC, D]
        qtb = np.zeros((D, QCOLS), np.float32)
        qtb[:, :S * BC] = qg.T
        m = dict(shared)
        m["qtb"] = qtb.astype(bf)
        m["qlast"] = np.ascontiguousarray(qg[(S - 1) * BC:, :].T)
        in_maps.append(m)
    return in_maps


def _install_ntff_shim():
    # Optional: enables NTFF hardware profiling under axon when tracing is
    # requested. Harmless no-op if the pieces are missing.
    import types, sys
    if "antenv.axon_hooks" in sys.modules:
        return
    try:
        import antenv
        from trn_agent_boot.trn_boot import _ntff_profile_via_ctypes
        hook = _ntff_profile_via_ctypes("/opt/axon/libaxon_pjrt.so")
        mod = types.ModuleType("antenv.axon_hooks")
        state = {"hook": hook}
        mod.get_axon_ntff_profile_hook = lambda: state["hook"]
        mod.set_axon_ntff_profile_hook = lambda h: state.update(hook=h)
        sys.modules["antenv.axon_hooks"] = mod
        antenv.axon_hooks = mod
    except Exception:
        pass


def kernel(**inputs) -> np.ndarray:
    if bool(int(os.environ.get("DKVMN_TRACE", "0"))):
        _install_ntff_shim()
    nc = _build_program()
    in_maps = _host_inputs(inputs)
    res = bass_utils.run_bass_kernel_spmd(
        nc, in_maps, core_ids=list(range(NCORES)),
        trace=bool(int(os.environ.get("DKVMN_TRACE", "0"))),
    )
    _CACHE["last_results"] = res
    pred = np.concatenate([res.results[c]["pred"].reshape(BC) for c in range(NCORES)])
    return pred.astype(np.float32)


# revision 49
# speedup vs baseline: 1.0161x; 1.0161x over previous
"""DKVMN forward kernel for 8 Trainium2 NeuronCores (v2).

Data-parallel over batch: B=128 -> 16 per core, split into 2 groups of
8 rows. Per-group state v [d=128 partitions, (m,b)=50*8=400 free] bf16.
(m,b) column order (col = m*8 + b) makes the e/a gate broadcasts have
stride-1 last dims so DVE tensor_tensor runs in 2x 16-bit mode.

Per group-step (slot s = 2t+g), engine assignment:
  w_ps  = sel_t.T @ attnb_g      PE (bf16) -> PSUM f32   (prefetch 2 slots)
  w_sb  = bf16(w_ps)             ACT copy
  z     = v * w_sb               DVE 2x
  zf    = z[:,:200]+z[:,200:]    GPSIMD (fold m 50->25)
  read  = reduce_m(zf) bf16      DVE (strided last dim, 1x)
  hps   = w1r@read + w1q@qe      PE
  th    = tanh(hps+b1) bf16      ACT
  e     = sigmoid(w2er@th+eb)    PE+ACT, bf16
  a     = tanh(w2ad@th+ab)       PE+ACT, bf16
  t1    = z * bc(e)              DVE 2x
  u     = (t1 * -1) + v          GPSIMD scalar_tensor_tensor
  t2    = w_sb * bc(a)           DVE 2x
  v'    = u + t2                 DVE 2x

Slot schedule "U(s-2); R(s); G(s)" keeps ~4 independent DVE ops queued
between a group's reduce and its gate-dependent t1, hiding the ACT/PE
gate latency.
"""

import os
import numpy as np
import ml_dtypes
from contextlib import ExitStack

import concourse.bass as bass
import concourse.bacc as bacc
import concourse.mybir as mybir
import concourse.tile as tile
import concourse.bass_utils as bass_utils
from concourse.masks import make_identity

B, S, M, D, NQ = 128, 100, 50, 128, 10000
NCORES = 8
BC = B // NCORES          # 16 batch rows per core
GB = 8                    # rows per group
GW = M * GB               # 400 group state width
HW = GW // 2              # 200 fold width
NQTILES = (S * BC + 127) // 128   # 13 gather tiles
QCOLS = NQTILES * 128     # 1664

F32 = mybir.dt.float32
BF16 = mybir.dt.bfloat16
I32 = mybir.dt.int32
AF = mybir.ActivationFunctionType
OP = mybir.AluOpType
AX = mybir.AxisListType

_CACHE = {}


def _build_program():
    if "nc" in _CACHE:
        return _CACHE["nc"]

    nc = bacc.Bacc("TRN2", target_bir_lowering=False, debug=False,
                   enable_asserts=False, num_devices=NCORES)

    dram_in = {}
    for name, shape, dt in [
        ("qtb", [D, QCOLS], BF16),
        ("qlast", [D, BC], F32),
        ("kTb", [D, M], BF16),
        ("w1r", [D, D], BF16), ("w1q", [D, D], BF16),
        ("w2er", [D, D], BF16), ("w2ad", [D, D], BF16),
        ("b1", [D, 1], F32), ("eb", [D, 1], F32), ("ab", [D, 1], F32),
        ("ow1r", [D, D], F32), ("ow1q", [D, D], F32),
        ("ob1", [D, 1], F32), ("ow2", [D, 1], F32), ("ob2", [1, 1], F32),
    ]:
        dram_in[name] = nc.dram_tensor(name, shape, dt, kind="ExternalInput").ap()
    pred_out = nc.dram_tensor("pred", [1, BC], F32, kind="ExternalOutput").ap()

    with tile.TileContext(nc) as tc, ExitStack() as ctx:
        persist = ctx.enter_context(tc.tile_pool(name="persist", bufs=1))

        # ---- persistent SBUF tiles ----
        kTb = persist.tile([D, M], BF16, tag="kTb")
        w1r = persist.tile([D, D], BF16, tag="w1r")
        w1q = persist.tile([D, D], BF16, tag="w1q")
        w2er = persist.tile([D, D], BF16, tag="w2er")
        w2ad = persist.tile([D, D], BF16, tag="w2ad")
        b1 = persist.tile([D, 1], F32, tag="b1")
        eb = persist.tile([D, 1], F32, tag="eb")
        ab = persist.tile([D, 1], F32, tag="ab")
        ow1r = persist.tile([D, D], F32, tag="ow1r")
        ow1q = persist.tile([D, D], F32, tag="ow1q")
        ob1 = persist.tile([D, 1], F32, tag="ob1")
        ow2 = persist.tile([D, 1], F32, tag="ow2")
        ob2 = persist.tile([1, 1], F32, tag="ob2")
        ident = persist.tile([128, 128], F32, tag="ident")
        identb = persist.tile([128, 128], BF16, tag="identb")
        qlast = persist.tile([D, BC], F32, tag="qlast")
        qTb = persist.tile([D, QCOLS], BF16, tag="qTb")
        attn = persist.tile([S, 2 * GW], F32, tag="attn")
        attnb = persist.tile([S, 2 * GW], BF16, tag="attnb")
        vpp = [[persist.tile([D, GW], BF16, name=f"v{g}p{p}", tag=f"v{g}p{p}")
                for p in (0, 1)] for g in (0, 1)]

        # qtb (the big one) split across three DGE queues; spread the
        # small weight loads across the other engines' DGE queues.
        nc.sync.dma_start(qTb[:], dram_in["qtb"][:])
        for i, (nm, t) in enumerate([
                ("kTb", kTb), ("w1r", w1r), ("w1q", w1q),
                ("w2er", w2er), ("w2ad", w2ad), ("b1", b1),
                ("eb", eb), ("ab", ab), ("ow1r", ow1r),
                ("ow1q", ow1q), ("ob1", ob1), ("ow2", ow2),
                ("ob2", ob2), ("qlast", qlast)]):
            eng = (nc.scalar, nc.gpsimd)[i % 2]
            eng.dma_start(t[:], dram_in[nm][:])
        make_identity(nc, ident[:])
        nc.vector.tensor_copy(identb[:], ident[:])
        nc.vector.memset(vpp[0][0][:], 0.0)
        nc.vector.memset(vpp[1][0][:], 0.0)

        # ---- phase 2: scores + softmax -> attn[s, (b,m)] f32 ----
        with tc.tile_pool(name="spsum", bufs=4, space="PSUM") as spsum:
            for b in range(BC):
                sc = spsum.tile([S, M], F32, tag="sc")
                qTsl = qTb[:, b:S * BC:BC]        # [128, 100] strided (s,b)
                nc.tensor.matmul(sc[:], qTsl, kTb[:], start=True, stop=True)
                if b % 2 == 0:
                    nc.vector.tensor_copy(attn[:, b * M:(b + 1) * M], sc[:])
                else:
                    nc.scalar.copy(attn[:, b * M:(b + 1) * M], sc[:])

        with tc.tile_pool(name="smx", bufs=1) as smx:
            a3 = attn[:].rearrange("p (b m) -> p b m", b=BC)
            mx = smx.tile([S, BC], F32, tag="mx")
            nc.vector.tensor_reduce(mx[:], a3, axis=AX.X, op=OP.max)
            mxb = mx[:, :, None].broadcast_to([S, BC, M])
            nc.vector.tensor_tensor(a3, a3, mxb, op=OP.subtract)
            nc.scalar.activation(attn[:], attn[:], AF.Exp)
            sm = smx.tile([S, BC], F32, tag="sm")
            nc.vector.tensor_reduce(sm[:], a3, axis=AX.X, op=OP.add)
            rec = smx.tile([S, BC], F32, tag="rec")
            nc.vector.reciprocal(rec[:], sm[:])
            recb = rec[:, :, None].broadcast_to([S, BC, M])
            nc.vector.tensor_tensor(a3, a3, recb, op=OP.mult)
            # reshuffle (b,m) f32 -> per-group (m,b) bf16
            src = attn[:].rearrange("p (g b m) -> p g b m", g=2, b=GB)
            dst = attnb[:].rearrange("p (g m b) -> p g b m", g=2, m=M)
            nc.vector.tensor_copy(dst, src)

        # ---- phase 3: the scan ----
        with tc.tile_pool(name="wps", bufs=3, space="PSUM") as wps, \
             tc.tile_pool(name="wsb", bufs=6) as wsbp, \
             tc.tile_pool(name="zp", bufs=3) as zp, \
             tc.tile_pool(name="zfp", bufs=3) as zfp, \
             tc.tile_pool(name="wide", bufs=8) as wide, \
             tc.tile_pool(name="small", bufs=16) as small, \
             tc.tile_pool(name="mlp", bufs=2, space="PSUM") as mlpp, \
             tc.tile_pool(name="fin", bufs=1, space="PSUM") as finp:

            NS = 2 * S  # slots
            FOLD_DVE = bool(int(os.environ.get("DKVMN_FOLD_DVE", "1")))
            STAGGER = bool(int(os.environ.get("DKVMN_STAGGER", "0")))
            wtile = [None] * NS     # w_sb bf16 tiles
            hqe = [None] * NS       # prefetched W1q@qe PSUM tiles
            state = [None] * NS     # (z, w_sb, e, a) per slot
            estore = [None] * NS    # e tiles kept for stagger dummies

            def emit_w(s):
                t, g = s // 2, s % 2
                sel = identb[0:S, t:t + 1].broadcast_to([S, D])
                wp = wps.tile([D, GW], F32, tag="wp")
                nc.tensor.matmul(wp[:], sel, attnb[:, g * GW:(g + 1) * GW],
                                 start=True, stop=True)
                w = wsbp.tile([D, GW], BF16, tag="w")
                nc.scalar.copy(w[:], wp[:])
                wtile[s] = w

            def emit_qe(s):
                t, g = s // 2, s % 2
                qeT = qTb[:, t * BC + g * GB: t * BC + (g + 1) * GB]
                gps = mlpp.tile([D, 3 * GB], F32, tag="hps")
                nc.tensor.matmul(gps[:, 0:GB], w1q[:], qeT, start=True,
                                 stop=False)
                hqe[s] = gps

            def emit_read_gates(s):
                t, g = s // 2, s % 2
                w = wtile[s]
                vcur = vpp[g][t % 2]
                if STAGGER and g == 1 and s + 1 < NS and \
                        estore[s + 1] is not None:
                    # Tiny op that reads group 0's NEXT-step e tile and
                    # rewrites one element of group 1's v unchanged
                    # (Identity(e*0 + v)): forces group 1's z to wait for
                    # group 0's step-(t+1) gates, staggering the two
                    # pipelines into anti-phase.
                    nc.scalar.activation(
                        vcur[0:1, 0:1], estore[s + 1][0:1, 0:1], AF.Identity,
                        bias=vcur[0:1, 0:1], scale=0.0)
                # z and the fused read+gate matmul in pipelined halves:
                # W1r @ read = sum_m (W1r @ z[:,m]) via stride-0 PSUM output
                # APs (start=False writes accumulate by address), with the
                # second z half computed while PE streams the first.
                z = zp.tile([D, GW], BF16, tag="z")
                gps = hqe[s]
                hqe[s] = None
                hbc = gps[:, None, 0:GB].broadcast_to([D, M // 2, GB])
                nc.vector.tensor_tensor(z[:, 0:HW], vcur[:, 0:HW],
                                        w[:, 0:HW], op=OP.mult)
                nc.tensor.matmul(
                    hbc, w1r[:],
                    z[:, 0:HW].rearrange("p (m b) -> p m b", m=M // 2),
                    start=False, stop=False)
                nc.vector.tensor_tensor(z[:, HW:GW], vcur[:, HW:GW],
                                        w[:, HW:GW], op=OP.mult)
                nc.tensor.matmul(
                    hbc, w1r[:],
                    z[:, HW:GW].rearrange("p (m b) -> p m b", m=M // 2),
                    start=False, stop=True)
                th = small.tile([D, GB], BF16, tag="th")
                nc.scalar.activation(th[:], gps[:, 0:GB], AF.Tanh, bias=b1[:])
                nc.tensor.matmul(gps[:, GB:2 * GB], w2er[:], th[:], start=True,
                                 stop=True)
                e = small.tile([D, GB], BF16, tag="e")
                nc.scalar.activation(e[:], gps[:, GB:2 * GB], AF.Sigmoid,
                                     bias=eb[:])
                nc.tensor.matmul(gps[:, 2 * GB:3 * GB], w2ad[:], th[:],
                                 start=True, stop=True)
                a = small.tile([D, GB], BF16, tag="a")
                nc.scalar.activation(a[:], gps[:, 2 * GB:3 * GB], AF.Tanh,
                                     bias=ab[:])
                state[s] = (z, w, e, a)
                estore[s] = e

            def emit_update(s):
                t, g = s // 2, s % 2
                z, w, e, a = state[s]
                vcur, vnext = vpp[g][t % 2], vpp[g][(t + 1) % 2]
                abc = a[:, None, :].broadcast_to([D, M // 2, GB])
                t2 = wide.tile([D, GW], BF16, tag="t2")
                q = wide.tile([D, GW], BF16, tag="q")
                for lo, hi in ((0, HW), (HW, GW)):
                    nc.vector.tensor_tensor(
                        t2[:, lo:hi].rearrange("p (m b) -> p m b", m=M // 2),
                        w[:, lo:hi].rearrange("p (m b) -> p m b", m=M // 2),
                        abc, op=OP.mult)
                    nc.vector.tensor_tensor(q[:, lo:hi], vcur[:, lo:hi],
                                            t2[:, lo:hi], op=OP.add)
                # t1 and v' in pipelined halves: v' of each half starts as
                # soon as its t1 half is done.
                ebc = e[:, None, :].broadcast_to([D, M // 2, GB])
                t1 = wide.tile([D, GW], BF16, tag="t1")
                for lo, hi in ((0, HW), (HW, GW)):
                    nc.vector.tensor_tensor(
                        t1[:, lo:hi].rearrange("p (m b) -> p m b", m=M // 2),
                        z[:, lo:hi].rearrange("p (m b) -> p m b", m=M // 2),
                        ebc, op=OP.mult)
                    nc.vector.tensor_tensor(vnext[:, lo:hi], q[:, lo:hi],
                                            t1[:, lo:hi], op=OP.subtract)
                state[s] = None
                if s < NS - 2:
                    wtile[s] = None    # keep the last step's w for the predict

            # Anti-phase slot order (0, 2, 1, 4, 3, ...): group 0 runs one
            # step ahead of group 1 so each group's gate latency is hidden
            # under the other group's DVE stretch instead of bunching.
            order = [0] + [x for k in range(1, S) for x in (2 * k, 2 * k - 1)] \
                + [NS - 1]
            emit_w(order[0]); emit_w(order[1])
            emit_qe(order[0])
            for i, s in enumerate(order):
                if s >= 2:
                    emit_update(s - 2)
                emit_read_gates(s)
                if i + 2 < NS:
                    emit_w(order[i + 2])
                if i + 1 < NS:
                    emit_qe(order[i + 1])
            emit_update(NS - 2)
            emit_update(NS - 1)

            # ---- final prediction (uses w from t=S-1, v after last update) ----
            readF = small.tile([D, BC], F32, tag="readF")
            for g in (0, 1):
                wf = wtile[2 * (S - 1) + g]
                zfin = zp.tile([D, GW], BF16, tag="z")
                nc.vector.tensor_tensor(zfin[:], vpp[g][S % 2][:], wf[:],
                                        op=OP.mult)
                nc.vector.tensor_reduce(
                    readF[:, g * GB:(g + 1) * GB],
                    zfin[:].rearrange("p (m b) -> p b m", b=GB),
                    axis=AX.X, op=OP.add)
            h2ps = finp.tile([D, BC], F32, tag="fin")
            nc.tensor.matmul(h2ps[:], ow1r[:], readF[:], start=True, stop=False)
            nc.tensor.matmul(h2ps[:], ow1q[:], qlast[:], start=False, stop=True)
            h2 = small.tile([D, BC], F32, tag="h2")
            nc.scalar.activation(h2[:], h2ps[:], AF.Relu, bias=ob1[:])
            pps = mlpp.tile([D, 3 * GB], F32, tag="hps")
            nc.tensor.matmul(pps[0:1, 0:BC], ow2[:], h2[:], start=True,
                             stop=True)
            ps = small.tile([1, BC], F32, tag="pred")
            nc.scalar.activation(ps[:], pps[0:1, 0:BC], AF.Sigmoid, bias=ob2[:])
            nc.sync.dma_start(pred_out[:], ps[:])

    nc.compile()
    _CACHE["nc"] = nc
    return nc


def _host_inputs(inputs):
    """Per-core input maps from the full problem inputs."""
    q = np.asarray(inputs["question_seq"]).astype(np.int64)
    emb = np.ascontiguousarray(np.asarray(inputs["emb"], dtype=np.float32))
    key_matrix = np.asarray(inputs["key_matrix"], dtype=np.float32)
    vu_w1 = np.asarray(inputs["vu_w1"], dtype=np.float32)
    vu_b1 = np.asarray(inputs["vu_b1"], dtype=np.float32)
    vu_w2 = np.asarray(inputs["vu_w2"], dtype=np.float32)
    vu_b2 = np.asarray(inputs["vu_b2"], dtype=np.float32)
    er_w = np.asarray(inputs["er_w"], dtype=np.float32)
    er_b = np.asarray(inputs["er_b"], dtype=np.float32)
    ad_w = np.asarray(inputs["ad_w"], dtype=np.float32)
    ad_b = np.asarray(inputs["ad_b"], dtype=np.float32)
    out_w1 = np.asarray(inputs["out_w1"], dtype=np.float32)
    out_b1 = np.asarray(inputs["out_b1"], dtype=np.float32)
    out_w2 = np.asarray(inputs["out_w2"], dtype=np.float32)
    out_b2 = np.asarray(inputs["out_b2"], dtype=np.float32)

    w2er = (vu_w2.astype(np.float64) @ er_w.astype(np.float64)).astype(np.float32)
    w2ad = (vu_w2.astype(np.float64) @ ad_w.astype(np.float64)).astype(np.float32)
    ebf = (vu_b2.astype(np.float64) @ er_w.astype(np.float64) + er_b).astype(np.float32)
    abf = (vu_b2.astype(np.float64) @ ad_w.astype(np.float64) + ad_b).astype(np.float32)

    bf = ml_dtypes.bfloat16
    shared = {
        "kTb": np.ascontiguousarray(key_matrix.T).astype(bf),
        "w1r": np.ascontiguousarray(vu_w1[:D]).astype(bf),
        "w1q": np.ascontiguousarray(vu_w1[D:]).astype(bf),
        "w2er": w2er.astype(bf), "w2ad": w2ad.astype(bf),
        "b1": vu_b1.reshape(D, 1), "eb": ebf.reshape(D, 1), "ab": abf.reshape(D, 1),
        "ow1r": np.ascontiguousarray(out_w1[:D]),
        "ow1q": np.ascontiguousarray(out_w1[D:]),
        "ob1": out_b1.reshape(D, 1),
        "ow2": np.ascontiguousarray(out_w2.reshape(D, 1)),
        "ob2": out_b2.reshape(1, 1),
    }
    in_maps = []
    for c in range(NCORES):
        qc = q[c * BC:(c + 1) * BC, :]          # [BC, S]
        idxs = qc.T.reshape(-1)                  # n = s*BC + b order
        qg = emb[idxs]                           # [S*BC, D]
        qtb = np.zeros((D, QCOLS), np.float32)
        qtb[:, :S * BC] = qg.T
        m = dict(shared)
        m["qtb"] = qtb.astype(bf)
        m["qlast"] = np.ascontiguousarray(qg[(S - 1) * BC:, :].T)
        in_maps.append(m)
    return in_maps


def _install_ntff_shim():
    # Optional: enables NTFF hardware profiling under axon when tracing is
    # requested. Harmless no-op if the pieces are missing.
    import types, sys
    if "antenv.axon_hooks" in sys.modules:
        return
    try:
        import antenv
        from trn_agent_boot.trn_boot import _ntff_profile_via_ctypes
        hook = _ntff_profile_via_ctypes("/opt/axon/libaxon_pjrt.so")
        mod = types.ModuleType("antenv.axon_hooks")
        state = {"hook": hook}
        mod.get_axon_ntff_profile_hook = lambda: state["hook"]
        mod.set_axon_ntff_profile_hook = lambda h: state.update(hook=h)
        sys.modules["antenv.axon_hooks"] = mod
        antenv.axon_hooks = mod
    except Exception:
        pass


def kernel(**inputs) -> np.ndarray:
    if bool(int(os.environ.get("DKVMN_TRACE", "0"))):
        _install_ntff_shim()
    nc = _build_program()
    in_maps = _host_inputs(inputs)
    res = bass_utils.run_bass_kernel_spmd(
        nc, in_maps, core_ids=list(range(NCORES)),
        trace=bool(int(os.environ.get("DKVMN_TRACE", "0"))),
    )
    _CACHE["last_results"] = res
    pred = np.concatenate([res.results[c]["pred"].reshape(BC) for c in range(NCORES)])
    return pred.astype(np.float32)


# revision 50
# speedup vs baseline: 1.0572x; 1.0405x over previous
"""DKVMN forward kernel for 8 Trainium2 NeuronCores (v2).

Data-parallel over batch: B=128 -> 16 per core, split into 2 groups of
8 rows. Per-group state v [d=128 partitions, (m,b)=50*8=400 free] bf16.
(m,b) column order (col = m*8 + b) makes the e/a gate broadcasts have
stride-1 last dims so DVE tensor_tensor runs in 2x 16-bit mode.

Per group-step (slot s = 2t+g), engine assignment:
  w_ps  = sel_t.T @ attnb_g      PE (bf16) -> PSUM f32   (prefetch 2 slots)
  w_sb  = bf16(w_ps)             ACT copy
  z     = v * w_sb               DVE 2x
  zf    = z[:,:200]+z[:,200:]    GPSIMD (fold m 50->25)
  read  = reduce_m(zf) bf16      DVE (strided last dim, 1x)
  hps   = w1r@read + w1q@qe      PE
  th    = tanh(hps+b1) bf16      ACT
  e     = sigmoid(w2er@th+eb)    PE+ACT, bf16
  a     = tanh(w2ad@th+ab)       PE+ACT, bf16
  t1    = z * bc(e)              DVE 2x
  u     = (t1 * -1) + v          GPSIMD scalar_tensor_tensor
  t2    = w_sb * bc(a)           DVE 2x
  v'    = u + t2                 DVE 2x

Slot schedule "U(s-2); R(s); G(s)" keeps ~4 independent DVE ops queued
between a group's reduce and its gate-dependent t1, hiding the ACT/PE
gate latency.
"""

import os
import numpy as np
import ml_dtypes
from contextlib import ExitStack

import concourse.bass as bass
import concourse.bacc as bacc
import concourse.mybir as mybir
import concourse.tile as tile
import concourse.bass_utils as bass_utils
from concourse.masks import make_identity

B, S, M, D, NQ = 128, 100, 50, 128, 10000
NCORES = 8
BC = B // NCORES          # 16 batch rows per core
GB = 8                    # rows per group
GW = M * GB               # 400 group state width
HW = GW // 2              # 200 fold width
NQTILES = (S * BC + 127) // 128   # 13 gather tiles
QCOLS = NQTILES * 128     # 1664

F32 = mybir.dt.float32
BF16 = mybir.dt.bfloat16
I32 = mybir.dt.int32
AF = mybir.ActivationFunctionType
OP = mybir.AluOpType
AX = mybir.AxisListType

_CACHE = {}


def _build_program():
    if "nc" in _CACHE:
        return _CACHE["nc"]

    nc = bacc.Bacc("TRN2", target_bir_lowering=False, debug=False,
                   enable_asserts=False, num_devices=NCORES)

    dram_in = {}
    for name, shape, dt in [
        ("qtb", [D, QCOLS], BF16),
        ("qlast", [D, BC], F32),
        ("kTb", [D, M], BF16),
        ("w1r", [D, D], BF16), ("w1q", [D, D], BF16),
        ("w2er", [D, D], BF16), ("w2ad", [D, D], BF16),
        ("b1", [D, 1], F32), ("eb", [D, 1], F32), ("ab", [D, 1], F32),
        ("ow1r", [D, D], F32), ("ow1q", [D, D], F32),
        ("ob1", [D, 1], F32), ("ow2", [D, 1], F32), ("ob2", [1, 1], F32),
    ]:
        dram_in[name] = nc.dram_tensor(name, shape, dt, kind="ExternalInput").ap()
    pred_out = nc.dram_tensor("pred", [1, BC], F32, kind="ExternalOutput").ap()

    with tile.TileContext(nc) as tc, ExitStack() as ctx:
        persist = ctx.enter_context(tc.tile_pool(name="persist", bufs=1))

        # ---- persistent SBUF tiles ----
        kTb = persist.tile([D, M], BF16, tag="kTb")
        w1r = persist.tile([D, D], BF16, tag="w1r")
        w1q = persist.tile([D, D], BF16, tag="w1q")
        w2er = persist.tile([D, D], BF16, tag="w2er")
        w2ad = persist.tile([D, D], BF16, tag="w2ad")
        b1 = persist.tile([D, 1], F32, tag="b1")
        eb = persist.tile([D, 1], F32, tag="eb")
        ab = persist.tile([D, 1], F32, tag="ab")
        ow1r = persist.tile([D, D], F32, tag="ow1r")
        ow1q = persist.tile([D, D], F32, tag="ow1q")
        ob1 = persist.tile([D, 1], F32, tag="ob1")
        ow2 = persist.tile([D, 1], F32, tag="ow2")
        ob2 = persist.tile([1, 1], F32, tag="ob2")
        ident = persist.tile([128, 128], F32, tag="ident")
        identb = persist.tile([128, 128], BF16, tag="identb")
        qlast = persist.tile([D, BC], F32, tag="qlast")
        qTb = persist.tile([D, QCOLS], BF16, tag="qTb")
        attn = persist.tile([S, 2 * GW], F32, tag="attn")
        attnb = persist.tile([S, 2 * GW], BF16, tag="attnb")
        vpp = [[persist.tile([D, GW], BF16, name=f"v{g}p{p}", tag=f"v{g}p{p}")
                for p in (0, 1)] for g in (0, 1)]

        # qtb (the big one) split across three DGE queues; spread the
        # small weight loads across the other engines' DGE queues.
        TH = QCOLS // 4
        nc.sync.dma_start(qTb[:, 0:2 * TH], dram_in["qtb"][:, 0:2 * TH])
        nc.scalar.dma_start(qTb[:, 2 * TH:3 * TH],
                            dram_in["qtb"][:, 2 * TH:3 * TH])
        nc.gpsimd.dma_start(qTb[:, 3 * TH:QCOLS],
                            dram_in["qtb"][:, 3 * TH:QCOLS])
        for i, (nm, t) in enumerate([
                ("kTb", kTb), ("w1r", w1r), ("w1q", w1q),
                ("w2er", w2er), ("w2ad", w2ad), ("b1", b1),
                ("eb", eb), ("ab", ab), ("ow1r", ow1r),
                ("ow1q", ow1q), ("ob1", ob1), ("ow2", ow2),
                ("ob2", ob2), ("qlast", qlast)]):
            eng = (nc.scalar, nc.gpsimd)[i % 2]
            eng.dma_start(t[:], dram_in[nm][:])
        make_identity(nc, ident[:])
        nc.vector.tensor_copy(identb[:], ident[:])
        nc.vector.memset(vpp[0][0][:], 0.0)
        nc.vector.memset(vpp[1][0][:], 0.0)

        # ---- phase 2: scores + softmax -> attn[s, (b,m)] f32 ----
        with tc.tile_pool(name="spsum", bufs=4, space="PSUM") as spsum:
            for b in range(BC):
                sc = spsum.tile([S, M], F32, tag="sc")
                qTsl = qTb[:, b:S * BC:BC]        # [128, 100] strided (s,b)
                nc.tensor.matmul(sc[:], qTsl, kTb[:], start=True, stop=True)
                if b % 2 == 0:
                    nc.vector.tensor_copy(attn[:, b * M:(b + 1) * M], sc[:])
                else:
                    nc.scalar.copy(attn[:, b * M:(b + 1) * M], sc[:])

        with tc.tile_pool(name="smx", bufs=1) as smx:
            a3 = attn[:].rearrange("p (b m) -> p b m", b=BC)
            mx = smx.tile([S, BC], F32, tag="mx")
            nc.vector.tensor_reduce(mx[:], a3, axis=AX.X, op=OP.max)
            mxb = mx[:, :, None].broadcast_to([S, BC, M])
            nc.vector.tensor_tensor(a3, a3, mxb, op=OP.subtract)
            nc.scalar.activation(attn[:], attn[:], AF.Exp)
            sm = smx.tile([S, BC], F32, tag="sm")
            nc.vector.tensor_reduce(sm[:], a3, axis=AX.X, op=OP.add)
            rec = smx.tile([S, BC], F32, tag="rec")
            nc.vector.reciprocal(rec[:], sm[:])
            recb = rec[:, :, None].broadcast_to([S, BC, M])
            nc.vector.tensor_tensor(a3, a3, recb, op=OP.mult)
            # reshuffle (b,m) f32 -> per-group (m,b) bf16
            src = attn[:].rearrange("p (g b m) -> p g b m", g=2, b=GB)
            dst = attnb[:].rearrange("p (g m b) -> p g b m", g=2, m=M)
            nc.vector.tensor_copy(dst, src)

        # ---- phase 3: the scan ----
        with tc.tile_pool(name="wps", bufs=3, space="PSUM") as wps, \
             tc.tile_pool(name="wsb", bufs=6) as wsbp, \
             tc.tile_pool(name="zp", bufs=3) as zp, \
             tc.tile_pool(name="zfp", bufs=3) as zfp, \
             tc.tile_pool(name="wide", bufs=8) as wide, \
             tc.tile_pool(name="small", bufs=16) as small, \
             tc.tile_pool(name="mlp", bufs=2, space="PSUM") as mlpp, \
             tc.tile_pool(name="fin", bufs=1, space="PSUM") as finp:

            NS = 2 * S  # slots
            FOLD_DVE = bool(int(os.environ.get("DKVMN_FOLD_DVE", "1")))
            STAGGER = bool(int(os.environ.get("DKVMN_STAGGER", "0")))
            wtile = [None] * NS     # w_sb bf16 tiles
            hqe = [None] * NS       # prefetched W1q@qe PSUM tiles
            state = [None] * NS     # (z, w_sb, e, a) per slot
            estore = [None] * NS    # e tiles kept for stagger dummies

            def emit_w(s):
                t, g = s // 2, s % 2
                sel = identb[0:S, t:t + 1].broadcast_to([S, D])
                wp = wps.tile([D, GW], F32, tag="wp")
                nc.tensor.matmul(wp[:], sel, attnb[:, g * GW:(g + 1) * GW],
                                 start=True, stop=True)
                w = wsbp.tile([D, GW], BF16, tag="w")
                nc.scalar.copy(w[:], wp[:])
                wtile[s] = w

            def emit_qe(s):
                t, g = s // 2, s % 2
                qeT = qTb[:, t * BC + g * GB: t * BC + (g + 1) * GB]
                gps = mlpp.tile([D, 3 * GB], F32, tag="hps")
                nc.tensor.matmul(gps[:, 0:GB], w1q[:], qeT, start=True,
                                 stop=False)
                hqe[s] = gps

            def emit_read_gates(s):
                t, g = s // 2, s % 2
                w = wtile[s]
                vcur = vpp[g][t % 2]
                if STAGGER and g == 1 and s + 1 < NS and \
                        estore[s + 1] is not None:
                    # Tiny op that reads group 0's NEXT-step e tile and
                    # rewrites one element of group 1's v unchanged
                    # (Identity(e*0 + v)): forces group 1's z to wait for
                    # group 0's step-(t+1) gates, staggering the two
                    # pipelines into anti-phase.
                    nc.scalar.activation(
                        vcur[0:1, 0:1], estore[s + 1][0:1, 0:1], AF.Identity,
                        bias=vcur[0:1, 0:1], scale=0.0)
                # z and the fused read+gate matmul in pipelined halves:
                # W1r @ read = sum_m (W1r @ z[:,m]) via stride-0 PSUM output
                # APs (start=False writes accumulate by address), with the
                # second z half computed while PE streams the first.
                z = zp.tile([D, GW], BF16, tag="z")
                gps = hqe[s]
                hqe[s] = None
                hbc = gps[:, None, 0:GB].broadcast_to([D, M // 2, GB])
                nc.vector.tensor_tensor(z[:, 0:HW], vcur[:, 0:HW],
                                        w[:, 0:HW], op=OP.mult)
                nc.tensor.matmul(
                    hbc, w1r[:],
                    z[:, 0:HW].rearrange("p (m b) -> p m b", m=M // 2),
                    start=False, stop=False)
                nc.vector.tensor_tensor(z[:, HW:GW], vcur[:, HW:GW],
                                        w[:, HW:GW], op=OP.mult)
                nc.tensor.matmul(
                    hbc, w1r[:],
                    z[:, HW:GW].rearrange("p (m b) -> p m b", m=M // 2),
                    start=False, stop=True)
                th = small.tile([D, GB], BF16, tag="th")
                nc.scalar.activation(th[:], gps[:, 0:GB], AF.Tanh, bias=b1[:])
                nc.tensor.matmul(gps[:, GB:2 * GB], w2er[:], th[:], start=True,
                                 stop=True)
                e = small.tile([D, GB], BF16, tag="e")
                nc.scalar.activation(e[:], gps[:, GB:2 * GB], AF.Sigmoid,
                                     bias=eb[:])
                nc.tensor.matmul(gps[:, 2 * GB:3 * GB], w2ad[:], th[:],
                                 start=True, stop=True)
                a = small.tile([D, GB], BF16, tag="a")
                nc.scalar.activation(a[:], gps[:, 2 * GB:3 * GB], AF.Tanh,
                                     bias=ab[:])
                state[s] = (z, w, e, a)
                estore[s] = e

            def emit_update(s):
                t, g = s // 2, s % 2
                z, w, e, a = state[s]
                vcur, vnext = vpp[g][t % 2], vpp[g][(t + 1) % 2]
                abc = a[:, None, :].broadcast_to([D, M, GB])
                t2 = wide.tile([D, GW], BF16, tag="t2")
                nc.vector.tensor_tensor(t2[:].rearrange("p (m b) -> p m b", m=M),
                                        w[:].rearrange("p (m b) -> p m b", m=M),
                                        abc, op=OP.mult)
                q = wide.tile([D, GW], BF16, tag="q")
                nc.vector.tensor_tensor(q[:], vcur[:], t2[:], op=OP.add)
                # t1 and v' in pipelined halves: v' of each half starts as
                # soon as its t1 half is done.
                ebc = e[:, None, :].broadcast_to([D, M // 2, GB])
                t1 = wide.tile([D, GW], BF16, tag="t1")
                for lo, hi in ((0, HW), (HW, GW)):
                    nc.vector.tensor_tensor(
                        t1[:, lo:hi].rearrange("p (m b) -> p m b", m=M // 2),
                        z[:, lo:hi].rearrange("p (m b) -> p m b", m=M // 2),
                        ebc, op=OP.mult)
                    nc.vector.tensor_tensor(vnext[:, lo:hi], q[:, lo:hi],
                                            t1[:, lo:hi], op=OP.subtract)
                state[s] = None
                if s < NS - 2:
                    wtile[s] = None    # keep the last step's w for the predict

            # Anti-phase slot order (0, 2, 1, 4, 3, ...): group 0 runs one
            # step ahead of group 1 so each group's gate latency is hidden
            # under the other group's DVE stretch instead of bunching.
            order = [0] + [x for k in range(1, S) for x in (2 * k, 2 * k - 1)] \
                + [NS - 1]
            emit_w(order[0]); emit_w(order[1])
            emit_qe(order[0])
            for i, s in enumerate(order):
                if s >= 2:
                    emit_update(s - 2)
                emit_read_gates(s)
                if i + 2 < NS:
                    emit_w(order[i + 2])
                if i + 1 < NS:
                    emit_qe(order[i + 1])
            emit_update(NS - 2)
            emit_update(NS - 1)

            # ---- final prediction (uses w from t=S-1, v after last update) ----
            readF = small.tile([D, BC], F32, tag="readF")
            for g in (0, 1):
                wf = wtile[2 * (S - 1) + g]
                zfin = zp.tile([D, GW], BF16, tag="z")
                nc.vector.tensor_tensor(zfin[:], vpp[g][S % 2][:], wf[:],
                                        op=OP.mult)
                nc.vector.tensor_reduce(
                    readF[:, g * GB:(g + 1) * GB],
                    zfin[:].rearrange("p (m b) -> p b m", b=GB),
                    axis=AX.X, op=OP.add)
            h2ps = finp.tile([D, BC], F32, tag="fin")
            nc.tensor.matmul(h2ps[:], ow1r[:], readF[:], start=True, stop=False)
            nc.tensor.matmul(h2ps[:], ow1q[:], qlast[:], start=False, stop=True)
            h2 = small.tile([D, BC], F32, tag="h2")
            nc.scalar.activation(h2[:], h2ps[:], AF.Relu, bias=ob1[:])
            pps = mlpp.tile([D, 3 * GB], F32, tag="hps")
            nc.tensor.matmul(pps[0:1, 0:BC], ow2[:], h2[:], start=True,
                             stop=True)
            ps = small.tile([1, BC], F32, tag="pred")
            nc.scalar.activation(ps[:], pps[0:1, 0:BC], AF.Sigmoid, bias=ob2[:])
            nc.sync.dma_start(pred_out[:], ps[:])

    nc.compile()
    _CACHE["nc"] = nc
    return nc


def _host_inputs(inputs):
    """Per-core input maps from the full problem inputs."""
    q = np.asarray(inputs["question_seq"]).astype(np.int64)
    emb = np.ascontiguousarray(np.asarray(inputs["emb"], dtype=np.float32))
    key_matrix = np.asarray(inputs["key_matrix"], dtype=np.float32)
    vu_w1 = np.asarray(inputs["vu_w1"], dtype=np.float32)
    vu_b1 = np.asarray(inputs["vu_b1"], dtype=np.float32)
    vu_w2 = np.asarray(inputs["vu_w2"], dtype=np.float32)
    vu_b2 = np.asarray(inputs["vu_b2"], dtype=np.float32)
    er_w = np.asarray(inputs["er_w"], dtype=np.float32)
    er_b = np.asarray(inputs["er_b"], dtype=np.float32)
    ad_w = np.asarray(inputs["ad_w"], dtype=np.float32)
    ad_b = np.asarray(inputs["ad_b"], dtype=np.float32)
    out_w1 = np.asarray(inputs["out_w1"], dtype=np.float32)
    out_b1 = np.asarray(inputs["out_b1"], dtype=np.float32)
    out_w2 = np.asarray(inputs["out_w2"], dtype=np.float32)
    out_b2 = np.asarray(inputs["out_b2"], dtype=np.float32)

    w2er = (vu_w2.astype(np.float64) @ er_w.astype(np.float64)).astype(np.float32)
    w2ad = (vu_w2.astype(np.float64) @ ad_w.astype(np.float64)).astype(np.float32)
    ebf = (vu_b2.astype(np.float64) @ er_w.astype(np.float64) + er_b).astype(np.float32)
    abf = (vu_b2.astype(np.float64) @ ad_w.astype(np.float64) + ad_b).astype(np.float32)

    bf = ml_dtypes.bfloat16
    shared = {
        "kTb": np.ascontiguousarray(key_matrix.T).astype(bf),
        "w1r": np.ascontiguousarray(vu_w1[:D]).astype(bf),
        "w1q": np.ascontiguousarray(vu_w1[D:]).astype(bf),
        "w2er": w2er.astype(bf), "w2ad": w2ad.astype(bf),
        "b1": vu_b1.reshape(D, 1), "eb": ebf.reshape(D, 1), "ab": abf.reshape(D, 1),
        "ow1r": np.ascontiguousarray(out_w1[:D]),
        "ow1q": np.ascontiguousarray(out_w1[D:]),
        "ob1": out_b1.reshape(D, 1),
        "ow2": np.ascontiguousarray(out_w2.reshape(D, 1)),
        "ob2": out_b2.reshape(1, 1),
    }
    in_maps = []
    for c in range(NCORES):
        qc = q[c * BC:(c + 1) * BC, :]          # [BC, S]
        idxs = qc.T.reshape(-1)                  # n = s*BC + b order
        qg = emb[idxs]                           # [S*BC, D]
        qtb = np.zeros((D, QCOLS), np.float32)
        qtb[:, :S * BC] = qg.T
        m = dict(shared)
        m["qtb"] = qtb.astype(bf)
        m["qlast"] = np.ascontiguousarray(qg[(S - 1) * BC:, :].T)
        in_maps.append(m)
    return in_maps


def _install_ntff_shim():
    # Optional: enables NTFF hardware profiling under axon when tracing is
    # requested. Harmless no-op if the pieces are missing.
    import types, sys
    if "antenv.axon_hooks" in sys.modules:
        return
    try:
        import antenv
        from trn_agent_boot.trn_boot import _ntff_profile_via_ctypes
        hook = _ntff_profile_via_ctypes("/opt/axon/libaxon_pjrt.so")
        mod = types.ModuleType("antenv.axon_hooks")
        state = {"hook": hook}
        mod.get_axon_ntff_profile_hook = lambda: state["hook"]
        mod.set_axon_ntff_profile_hook = lambda h: state.update(hook=h)
        sys.modules["antenv.axon_hooks"] = mod
        antenv.axon_hooks = mod
    except Exception:
        pass


def kernel(**inputs) -> np.ndarray:
    if bool(int(os.environ.get("DKVMN_TRACE", "0"))):
        _install_ntff_shim()
    nc = _build_program()
    in_maps = _host_inputs(inputs)
    res = bass_utils.run_bass_kernel_spmd(
        nc, in_maps, core_ids=list(range(NCORES)),
        trace=bool(int(os.environ.get("DKVMN_TRACE", "0"))),
    )
    _CACHE["last_results"] = res
    pred = np.concatenate([res.results[c]["pred"].reshape(BC) for c in range(NCORES)])
    return pred.astype(np.float32)
